# revision 88
# baseline (speedup 1.0000x reference)
"""Trainium2 Bass kernel for AdaBiDiff GNN message passing.

Per-core computation (data parallel over batch B=8, one batch element per core):
  xt (12,1536); ex = exp(xt); S = colsum(ex); L = ln S
  Ghat_S[i,j] = sum_t ex[t,i] x[t,j] + crow[i] - S[i] L[j]
              = S[i]*(0.5 - kl[i,j]),  crow = S*(0.5+L) - colsum(ex*x)
  A = (Ghat_S > 0);  u_fwd = (A @ xt.T)/rowsum(A);  u_bwd = (A.T @ xt.T)/colsum(A)
  v = (3/7) u_fwd + u_bwd;  zT = relu(bdW1.x + bd(2.1 W2).v) via one stacked
  matmul; two MLP blocks (BN folded on host, bf16) -> out (12,1536).

Implementation notes:
  - P1 = [ex(0:12); 0(12:32); crow(32); S(33)], Q = [x(0:12); junk; 1(32);
    -L(33)], both duplicated at partitions 64.. for the transposed Ghat
    orientation (tile_position (64,0)).  Engine-op APs must start at
    32-aligned partitions; S and -L rows are staged through base-0 rows and
    DMA'd into place.
  - orientation-2 compare on ACT (Sign -> -1/0/1), orientation-1 on DVE
    (is_gt -> 0/1); the sign-affine fix is folded into u_fwd's denominator:
    uf = (y' + Sx)/(rs' + N).
  - A tiles in fp8e4, paired over i-blocks: products run MatmulPerfMode
    DoubleRow (256-deep contraction, 0.5 cyc/row; stationary <= 32 out
    partitions, no col tile_position).  xtT stationary is fp8 with a hi+lo
    column split; hi and lo product groups accumulate into the same PSUM
    rows, and the ones column sits at col 0 so rowsums land on partition 0.
  - stage B+C run as a per-512-column-chunk pipeline: one chunk's products
    need 2 PSUM banks, leaving 4 for Ghat double-buffering, and chunk c's
    scaling overlaps chunk c+1's compares.
  - zT moving is [x; junk; v] vs stationary [bdW1; 0; bd(2.1W2)] - zero
    weight rows null the junk rows.  MLP weights ship in one bf16 DRAM pack
    (single DMA); activations bf16; MLP layers run layer-outer over 512
    chunks so chunk c+1's matmuls overlap chunk c's relu tail.
"""

import os
import numpy as np

import concourse.bass as bass
import concourse.bacc as bacc
import concourse.tile as tile
import concourse.mybir as mybir

F32 = mybir.dt.float32
F32R = mybir.dt.float32r
BF16 = mybir.dt.bfloat16
FP8 = mybir.dt.float8e4
AF = mybir.ActivationFunctionType
ALU = mybir.AluOpType
DR = mybir.MatmulPerfMode.DoubleRow

B, T, N, H, TH, HID2, TOUT = 8, 12, 1536, 64, 768, 128, 12
NT = N // 128
NC = N // 512
NC2 = 2            # 1024 + 512 chunking for bf16 moving
CH2 = [slice(0, 1024), slice(1024, 1536)]
CW2 = [1024, 512]

# weight-pack column offsets (bf16, 128 partitions)
O_EW1, O_EPROJ, O_EW2, O_EW3, O_DW2, O_DW1, O_DPROJ = (
    0, 768, 1152, 1280, 1344, 1472, 1600)
F2 = 1632

_cache = {}


def _build_nc():
    nc = bacc.Bacc("TRN2", target_bir_lowering=False, debug=False)
    d = {}

    def dp(name, shape, dt=F32R, out=False):
        d[name] = nc.declare_dram_parameter(name, list(shape), dt, isOutput=out)

    dp("x", (T, N))
    dp("wpack", (128, F2), BF16)
    dp("wz", (45, TH))
    dp("bias", (128, 6), F32)
    dp("i12", (T, T))
    dp("out", (T, N), F32, out=True)

    with tile.TileContext(nc) as tc:
        _kernel_body(tc, d)
    nc.compile()
    return nc


def _kernel_body(tc, d):
    nc = tc.nc
    CS = [slice(c * 512, (c + 1) * 512) for c in range(NC)]

    with tc.tile_pool(name="w", bufs=1) as w, tc.tile_pool(name="sb", bufs=1) as sb:

        def stile(name, shape, dt=F32R):
            return sb.tile(list(shape), dt, name=name, tag=name)

        # ---- inputs / weights (x and i12 first: they gate stage A) ----
        xt = stile("xt", (T, N))
        nc.sync.dma_start(out=xt[:], in_=d["x"].ap())
        i12 = w.tile([T, T], F32R, name="i12", tag="i12")
        nc.sync.dma_start(out=i12[:], in_=d["i12"].ap())

        wp = w.tile([128, F2], BF16, name="wp", tag="wp")
        nc.sync.dma_start(out=wp[:], in_=d["wpack"].ap())
        wz = w.tile([45, TH], F32R, name="wz", tag="wz")
        nc.sync.dma_start(out=wz[:], in_=d["wz"].ap())
        bias = w.tile([128, 6], F32, name="bias", tag="bias")
        nc.sync.dma_start(out=bias[:], in_=d["bias"].ap())

        ew1 = wp[:, O_EW1:O_EPROJ]
        eproj = wp[:, O_EPROJ:O_EW2]
        ew2 = wp[:, O_EW2:O_EW3]
        ew3 = wp[:, O_EW3:O_DW2]
        dw2 = wp[:, O_DW2:O_DW1]
        dw1 = wp[0:H, O_DW1:O_DPROJ]
        dproj = wp[0:H, O_DPROJ:O_DPROJ + TOUT]
        dw3 = wp[0:HID2, O_DPROJ + TOUT:O_DPROJ + 2 * TOUT]
        eb1 = bias[0:HID2, 0:1]
        eb2 = bias[0:HID2, 1:2]
        ebe = bias[0:H, 2:3]
        db1 = bias[0:HID2, 3:4]
        db2 = bias[0:HID2, 4:5]
        dbd = bias[0:TOUT, 5:6]

        ones12 = w.tile([T, 1], F32R, name="ones12", tag="ones12")
        nc.vector.memset(ones12[:].bitcast(F32), 1.0)
        # preload the one activation table that covers exp+ln+relu+sign+
        # identity (act_func_sets[6] = natural_log_exp_and_others) under the
        # input DMAs, so the insertion pass emits no further loads
        nc.scalar.add_instruction(mybir.InstLoadActFuncSet(
            name=nc.get_next_instruction_name(), act_func_set_id=6,
            ins=[], outs=[]))

        # =========== Stage A (S-form, no normalization) ===========
        # P1 = [ex(12); crow; S] and Q = [x(12); 1; -L; v(12)], each
        # duplicated at partitions 64.. for the transposed-orientation
        # quadrant.  Ghat_S[i,j] = sum_t ex[t,i] x[t,j] + crow[i] - S[i] L[j]
        # = S[i]*(0.5 - kl[i,j]) with crow = S*(0.5 + L) - colsum(ex*x).
        P1 = stile("P1", (98, N))
        Q = stile("Q", (98, N))
        M = stile("M", (45, N))
        xtT = stile("xtT", (128, NT, 64), FP8)
        Srow = stile("Srow", (1, N), F32)
        Sx = stile("Sx", (T, 1), F32)
        Lrow = stile("Lrow", (1, N), F32)
        t2 = stile("t2", (1, N), F32)
        q2 = stile("q2", (T, N))

        # P1 rows 12:32 must be exact zero (they pair Q's junk rows); Q row
        # 32 keeps the memset 1.0 (ones row), rows 12:32 are junk.  P1 first:
        # it write-after-write gates the exp.
        nc.gpsimd.memset(P1[0:64, :].bitcast(F32), 0.0)
        nc.gpsimd.memset(Q[0:64, :].bitcast(F32), 1.0)
        nc.gpsimd.memset(xtT[:], 0.0)
        nc.sync.dma_start(out=Q[0:T, :], in_=d["x"].ap())
        nc.gpsimd.memset(M[0:45, :].bitcast(F32), 0.0)
        nc.sync.dma_start(out=M[0:T, :], in_=d["x"].ap())
        nc.vector.tensor_reduce(Sx[:], xt[:].bitcast(F32), mybir.AxisListType.X,
                                ALU.add)
        SxN = stile("SxN", (13, 1), F32)
        nc.vector.memset(SxN[0:1, :], float(N))
        nc.sync.dma_start(out=SxN[1:13, :], in_=Sx[:])

        ex = P1[0:T, :].bitcast(F32)

        with tc.tile_pool(name="pa", bufs=2, space="PSUM") as pa, \
             tc.tile_pool(name="pat", bufs=1, space="PSUM") as pat:
            # per-chunk chain: exp -> S -> ln -> t2 -> crow (S-copy and q2
            # are off-chain: ACT and Pool respectively)
            for c in range(NC):
                nc.scalar.activation(P1[0:T, CS[c]], xt[:, CS[c]].bitcast(F32), AF.Exp)
                psS = pa.tile([1, 512], F32, name="psS", tag="pa")
                nc.tensor.matmul(psS[:], ones12[:], P1[0:T, CS[c]], start=True,
                                 stop=True)
                nc.scalar.activation(Lrow[:, CS[c]], psS[:], AF.Ln)
                nc.scalar.activation(Srow[:, CS[c]], psS[:], AF.Identity,
                                     scale=-1.0)
                nc.sync.dma_start(out=P1[33:34, CS[c]], in_=Srow[:, CS[c]].bitcast(F32R))
                nc.gpsimd.tensor_tensor(q2[:, CS[c]], ex[:, CS[c]],
                                        xt[:, CS[c]].bitcast(F32), ALU.mult)
                psE = pa.tile([1, 512], F32, name="psE", tag="pa")
                nc.tensor.matmul(psE[:], ones12[:], q2[:, CS[c]],
                                 start=True, stop=True)
                # t2 = (L + 0.5) * S;  crow = t2 - E2
                nc.vector.scalar_tensor_tensor(
                    t2[:, CS[c]], Lrow[:, CS[c]], 0.5, psS[:],
                    op0=ALU.add, op1=ALU.mult)
                nc.vector.tensor_tensor(P1[32:33, CS[c]], t2[:, CS[c]],
                                        psE[:], ALU.subtract)
                nc.sync.dma_start(out=Q[33:34, CS[c]], in_=Lrow[:, CS[c]].bitcast(F32R))
                # per-chunk dup at partitions 64.. lets the transposed
                # orientation start before the whole row is finished
                nc.sync.dma_start(out=P1[64:98, CS[c]], in_=P1[0:34, CS[c]])
                nc.sync.dma_start(out=Q[64:98, CS[c]], in_=Q[0:34, CS[c]])

            # transposed x with fp8 hi/lo split and ones column
            psT = pat.tile([128, NT, T], F32, name="psT", tag="psT")
            for j in range(NT):
                nc.tensor.matmul(psT[:, j, :], xt[:, j * 128:(j + 1) * 128], i12[:],
                                 start=True, stop=True)
            # cols [x_hi(12) | ones | pad(3) | x_lo(12) | 0 | pad(3)]: the hi
            # and lo product groups accumulate into the same PSUM rows.
            nc.vector.tensor_copy(xtT[:, :, 1:1 + T], psT[:])
            nc.vector.tensor_tensor(xtT[:, :, 33:33 + T], psT[:], xtT[:, :, 1:1 + T],
                                    ALU.subtract)
            nc.vector.memset(xtT[:, :, 0:1], 1.0)

        # =========== Stages B+C: per-512-chunk pipeline ===========
        # Products for one chunk need only 2 PSUM banks, freeing 4 for deep
        # Ghat double-buffering; chunk c's scaling (stage C) overlaps chunk
        # c+1's Ghat/compares.
        rr = stile("rr", (1, N))
        cc = stile("cc", (1, N))
        uf = stile("uf", (13, N), F32)
        ub = stile("ub", (13, N), F32)
        y3 = stile("y3", (13, N))
        b3 = stile("b3", (13, N))
        ones13 = w.tile([1, 13], F32R, name="ones13", tag="ones13")
        nc.vector.memset(ones13[:].bitcast(F32), 1.0)
        s37 = w.tile([1, 13], F32R, name="s37", tag="s37")
        nc.vector.memset(s37[:].bitcast(F32), 3.0 / 7.0)
        rrBs = stile("rrBs", (13, N), F32)
        ccBs = stile("ccBs", (13, N), F32)

        zT = stile("zT", (128, 6, N), BF16)
        h1 = stile("h1", (HID2, N), BF16)
        h2 = stile("h2", (HID2, N), BF16)
        xe = stile("xe", (H, N), BF16)
        g1 = stile("g1", (HID2, N), BF16)
        g2 = stile("g2", (HID2, N), BF16)
        od = stile("od", (TOUT, N), F32)
        nlc = [0]

        def nl(out, ps, bias, relu):
            # alternate the nonlinearity between ACT and DVE
            n = nlc[0]; nlc[0] += 1
            if n % 2 == 0:
                func = AF.Relu if relu else AF.Identity
                if bias is None:
                    nc.scalar.activation(out, ps, func)
                else:
                    nc.scalar.activation(out, ps, func, bias=bias)
            elif relu:
                if bias is None:
                    nc.vector.tensor_scalar(out, ps, 0.0, None, ALU.max)
                else:
                    nc.vector.tensor_scalar(out, ps, bias, 0.0, ALU.add, ALU.max)
            else:
                nc.vector.tensor_scalar(out, ps, bias, None, ALU.add)

        with tc.tile_pool(name="pp", bufs=1, space="PSUM") as pp, \
             tc.tile_pool(name="pga", bufs=4, space="PSUM") as pga, \
             tc.tile_pool(name="pc", bufs=1, space="PSUM") as pcp, \
             tc.tile_pool(name="ab", bufs=3) as ab:
            for c in range(NC):
                prodF = pp.tile([32, 512], F32, name="prodF", tag="prodF")
                prodB = pp.tile([32, 512], F32, name="prodB", tag="prodB")
                for m in range(NT // 2):
                    A2b = ab.tile([128, 2, 512], FP8, name="A2b", tag="A2b")
                    A2f = ab.tile([128, 2, 512], FP8, name="A2f", tag="A2f")
                    for s in range(2):
                        i = 2 * m + s
                        isl = slice(i * 128, (i + 1) * 128)
                        psA = pga.tile([128, 512], F32, name="psA", tag="psG")
                        nc.tensor.matmul(psA[:], P1[0:34, isl], Q[0:34, CS[c]],
                                         start=True, stop=True, tile_position=(0, 0))
                        nc.vector.tensor_scalar(A2b[:, s, :], psA[:], 0.0,
                                                None, ALU.is_gt)
                        psB = pga.tile([128, 512], F32, name="psB", tag="psG")
                        nc.tensor.matmul(psB[:], Q[64:98, isl], P1[64:98, CS[c]],
                                         start=True, stop=True, tile_position=(64, 0))
                        nc.scalar.sign(A2f[:, s, :], psB[:])
                    for g in range(2):
                        gsl = slice(32 * g, 32 * g + 32)
                        st = (m == 0 and g == 0)
                        sp = (m == NT // 2 - 1 and g == 1)
                        nc.tensor.matmul(prodF[:, :],
                                         xtT[:, 2 * m:2 * m + 2, gsl],
                                         A2f[:, :, :], start=st, stop=sp,
                                         perf_mode=DR, skip_group_check=True)
                        nc.tensor.matmul(prodB[:, :],
                                         xtT[:, 2 * m:2 * m + 2, gsl],
                                         A2b[:, :, :], start=st, stop=sp,
                                         perf_mode=DR, skip_group_check=True)

                # ==== stage C for this chunk ====
                # y3 row 0 = rs' + N (sign-affine denom), rows 1:13 = y + Sx
                with nc.allow_low_precision(reason="4-byte recips"):
                    nc.scalar.activation(y3[:, CS[c]], prodF[0:13, :],
                                         AF.Identity, bias=SxN[:])
                    nc.vector.reciprocal(rr[:, CS[c]], y3[0:1, CS[c]])
                    nc.scalar.activation(b3[:, CS[c]], prodB[0:13, :],
                                         AF.Identity)
                    nc.vector.reciprocal(cc[:, CS[c]], b3[0:1, CS[c]])
                rrB = pcp.tile([13, 512], F32, name="rrB", tag="rrB")
                nc.tensor.matmul(rrB[:], s37[:], rr[:, CS[c]],
                                 start=True, stop=True)
                nc.scalar.activation(rrBs[:, CS[c]], rrB[:], AF.Identity)
                nc.gpsimd.tensor_tensor(uf[:, CS[c]], y3[:, CS[c]].bitcast(F32),
                                        rrBs[:, CS[c]], ALU.mult)
                ccB = pcp.tile([13, 512], F32, name="ccB", tag="ccB")
                nc.tensor.matmul(ccB[:], ones13[:], cc[:, CS[c]],
                                 start=True, stop=True)
                nc.scalar.activation(ccBs[:, CS[c]], ccB[:], AF.Identity)
                nc.gpsimd.tensor_tensor(ub[:, CS[c]], b3[:, CS[c]].bitcast(F32),
                                        ccBs[:, CS[c]], ALU.mult)
                # v = (3/7)uf + ub (3/7 folded into rrB) -> M[32:45]
                # (row 32 junk, killed by the zero Wz row)
                nc.gpsimd.tensor_tensor(M[32:45, CS[c]], uf[:, CS[c]],
                                        ub[:, CS[c]], ALU.add)



        # =========== Stages D/E/F ===========
        # Layer-outer over 512-column chunks: chunk c+1's matmuls overlap
        # chunk c's relu/bias tail.
        with tc.tile_pool(name="pf", bufs=4, space="PSUM") as pf:
            for c in range(NC):
                for k in range(6):
                    ps = pf.tile([128, 512], F32, name="psF", tag="ps")
                    nc.tensor.matmul(ps[:], wz[:, k * 128:(k + 1) * 128],
                                     M[0:45, CS[c]], start=True, stop=True)
                    nl(zT[:, k, CS[c]], ps[:], None, True)
            psXe = []
            for c in range(NC):
                ps = pf.tile([H, 512], F32, name="psXe", tag="psXe")
                psXe.append(ps)
                for k in range(6):
                    nc.tensor.matmul(ps[:], eproj[:, k * 64:(k + 1) * 64],
                                     zT[:, k, CS[c]], start=(k == 0), stop=False,
                                     skip_group_check=True)
            for c in range(NC):
                ps = pf.tile([HID2, 512], F32, name="psH1", tag="ps")
                for k in range(6):
                    nc.tensor.matmul(ps[:], ew1[:, k * 128:(k + 1) * 128],
                                     zT[:, k, CS[c]], start=(k == 0), stop=(k == 5))
                nl(h1[:, CS[c]], ps[:], eb1, True)
            for c in range(NC):
                ps = pf.tile([HID2, 512], F32, name="psH2", tag="ps")
                nc.tensor.matmul(ps[:], ew2[:], h1[:, CS[c]], start=True, stop=True)
                nl(h2[:, CS[c]], ps[:], eb2, True)
            for c in range(NC):
                nc.tensor.matmul(psXe[c][:], ew3[:], h2[:, CS[c]], start=False,
                                 stop=True, skip_group_check=True)
                nl(xe[:, CS[c]], psXe[c][:], ebe, False)
            for c in range(NC):
                ps = pf.tile([HID2, 512], F32, name="psG1", tag="ps")
                nc.tensor.matmul(ps[:], dw1[:], xe[:, CS[c]], start=True, stop=True)
                nl(g1[:, CS[c]], ps[:], db1, True)
            for c in range(NC):
                ps = pf.tile([HID2, 512], F32, name="psG2", tag="ps")
                nc.tensor.matmul(ps[:], dw2[:], g1[:, CS[c]], start=True, stop=True)
                nl(g2[:, CS[c]], ps[:], db2, True)
            for c in range(NC):
                ps = pf.tile([TOUT, 512], F32, name="psOd", tag="ps")
                nc.tensor.matmul(ps[:], dw3[:], g2[:, CS[c]], start=True, stop=False)
                nc.tensor.matmul(ps[:], dproj[:], xe[:, CS[c]], start=False, stop=True)
                nl(od[:, CS[c]], ps[:], dbd, False)
                nc.sync.dma_start(out=d["out"].ap()[:, CS[c]], in_=od[:, CS[c]])


def _host_weights(inputs):
    f32 = np.float32
    import ml_dtypes
    bf16 = ml_dtypes.bfloat16
    W1 = np.asarray(inputs["W1"], f32)[0]
    W2 = np.asarray(inputs["W2"], f32)[0]

    # stacked zT stationary: [bdW1(12); 0; 0; bdW2'(12)] with W2' = 2.1*W2
    wzm = np.zeros((45, TH), f32)
    for t in range(T):
        wzm[t, t * H:(t + 1) * H] = W1
        wzm[33 + t, t * H:(t + 1) * H] = 2.1 * W2

    g = np.asarray(inputs["enc_bn_g"], f32); be = np.asarray(inputs["enc_bn_b"], f32)
    m = np.asarray(inputs["enc_bn_m"], f32); v = np.asarray(inputs["enc_bn_v"], f32)
    esc = g / np.sqrt(v + 1e-5)
    ew3 = np.asarray(inputs["enc_w3"], f32) * esc[None, :]
    eproj = np.asarray(inputs["enc_proj"], f32) * esc[None, :]
    ebe = np.asarray(inputs["enc_b3"], f32) * esc + (be - m * esc)
    g = np.asarray(inputs["dec_bn_g"], f32); bd = np.asarray(inputs["dec_bn_b"], f32)
    m = np.asarray(inputs["dec_bn_m"], f32); v = np.asarray(inputs["dec_bn_v"], f32)
    dsc = g / np.sqrt(v + 1e-5)
    dw3 = np.asarray(inputs["dec_w3"], f32) * dsc[None, :]
    dproj = np.asarray(inputs["dec_proj"], f32) * dsc[None, :]
    dbd = np.asarray(inputs["dec_b3"], f32) * dsc + (bd - m * dsc)

    wpack = np.zeros((128, F2), f32)
    ew1 = np.asarray(inputs["enc_w1"], f32).reshape(6, 128, HID2)
    wpack[:, O_EW1:O_EPROJ] = ew1.transpose(1, 0, 2).reshape(128, 768)
    epr = eproj.reshape(6, 128, H)
    wpack[:, O_EPROJ:O_EW2] = epr.transpose(1, 0, 2).reshape(128, 384)
    wpack[:, O_EW2:O_EW3] = np.asarray(inputs["enc_w2"], f32)
    wpack[:, O_EW3:O_DW2] = ew3
    wpack[:, O_DW2:O_DW1] = np.asarray(inputs["dec_w2"], f32)
    wpack[0:H, O_DW1:O_DPROJ] = np.asarray(inputs["dec_w1"], f32)
    wpack[0:H, O_DPROJ:O_DPROJ + TOUT] = dproj
    wpack[0:HID2, O_DPROJ + TOUT:O_DPROJ + 2 * TOUT] = dw3

    biasm = np.zeros((128, 6), f32)
    biasm[0:HID2, 0] = np.asarray(inputs["enc_b1"], f32)
    biasm[0:HID2, 1] = np.asarray(inputs["enc_b2"], f32)
    biasm[0:H, 2] = ebe
    biasm[0:HID2, 3] = np.asarray(inputs["dec_b1"], f32)
    biasm[0:HID2, 4] = np.asarray(inputs["dec_b2"], f32)
    biasm[0:TOUT, 5] = dbd

    return {
        "wpack": np.ascontiguousarray(wpack.astype(bf16)),
        "wz": wzm,
        "bias": biasm,
        "i12": np.eye(T, dtype=f32),
    }


def make_in_maps(inputs):
    wmap = _host_weights(inputs)
    x = np.asarray(inputs["x"], np.float32)
    in_maps = []
    for b in range(B):
        m = dict(wmap)
        m["x"] = np.ascontiguousarray(x[b, :, :, 0])
        in_maps.append(m)
    return in_maps


def kernel(**inputs) -> np.ndarray:
    from concourse.bass_utils import run_bass_kernel_spmd

    if "nc" not in _cache:
        _cache["nc"] = _build_nc()
    nc = _cache["nc"]

    in_maps = make_in_maps(inputs)
    trace = bool(int(os.environ.get("KERNEL_TRACE", "0")))
    res = run_bass_kernel_spmd(nc, in_maps, core_ids=list(range(B)), trace=trace)
    _cache["last_result"] = res
    out = np.stack([np.asarray(res.results[b]["out"]) for b in range(B)], axis=0)
    return out[..., None].astype(np.float32)
